# revision 1
# baseline (speedup 1.0000x reference)
"""Causal multi-head attention kernel for Trainium2 (8 NeuronCores).

Problem: x[1,2048,1024] -> qkv proj (W_qkv[1024,3072]) -> 64 heads of dim 16
         -> causal softmax attention -> out proj (W_out[1024,1024]).

Sharding: Megatron-style head parallelism. Each of the 8 cores owns 8 heads
(a 128-wide column slice of W_qkv per q/k/v and a 128-row slice of W_out),
computes a partial output projection, and the host sums the 8 partials
(the "all-reduce").

Per-core pipeline (all matmuls bf16 with fp32 PSUM accumulation):
  1. x -> SBUF, cast bf16, DMA-xbar transpose to xT [c, t] layout.
  2. qT/kT = (W q/k slice, head-spaced 16-per-32-partitions).T @ xT so that
     4 heads sit at partition offsets 0/32/64/96 (enables PE tile_position
     row-packing of the K=16 score matmuls).  v in natural [t, c] layout with
     a ones column appended per head (fused softmax denominator).
  3. Flash-style attention per 4-head group and 512-query block:
     S^T tiles = kT.T @ qT (4 row-packed matmuls -> 2-bank PSUM sets),
     exp via one ScalarE activation per 2-bank set (scale=1/4, no max
     subtraction -- scores are O(1) by construction), causal masking on the
     bf16 exp tiles (triangle multiply + memset), PV accumulation via 4
     col-packed matmuls into one PSUM bank ([17,512] per head: 16 dims +
     rowsum).  1/rowsum via ACT ln + exp(-x) (same table set as Exp... Ln/Exp).
  4. Partial out-projection y_p = attnT.T @ W_out_slice (+ b_out on core 0).

Self-contained: hardcodes all shapes; host code only slices inputs per core
and sums the 8 partial outputs.
"""

import numpy as np
from contextlib import ExitStack

import ml_dtypes

import concourse.bass as bass
import concourse.tile as tile
from concourse import mybir
from concourse.bass_utils import run_bass_kernel_spmd

F32 = mybir.dt.float32
BF16 = mybir.dt.bfloat16
AF = mybir.ActivationFunctionType

T = 2048
C = 1024
HDIM = 16
NHEADS = 64
NCORES = 8
HPC = NHEADS // NCORES      # 8 heads per core
CSLICE = HPC * HDIM         # 128 channel slice per core
G = 2                       # head groups of 4 per core
NCH = C // 128              # 8 contraction chunks
NT = T // 128               # 16 token chunks of 128
NQ = T // 512               # 4 query blocks of 512

_CACHE = {}


def _legalize_waits(nc):
    """This neuronxcc/walrus build encodes at most ONE sync-wait per
    instruction (two on EventSemaphore) — multi-wait sync_info dies in
    codegen with "Too many sync wait commands".  Hoist excess waits into
    standalone EventSemaphore instructions on the same engine immediately
    before the instruction (engine queues are in-order, so semantics are
    preserved)."""
    import bass_rust
    n = 0
    for f in nc.m.functions:
        for blk in f.blocks:
            out = []
            changed = False
            for inst in blk.instructions:
                si = inst.sync_info
                waits = list(si.on_wait) if si is not None and si.on_wait else []
                cap = 2 if isinstance(inst, mybir.InstEventSemaphore) else 1
                if len(waits) > cap:
                    extra, keep = waits[:-cap], waits[-cap:]
                    for i in range(0, len(extra), 2):
                        ev = mybir.InstEventSemaphore(
                            name=f"evwait-{n}", ins=[], outs=[])
                        n += 1
                        ev.engine = inst.engine
                        ev.sync_info = bass_rust.SyncInfo(
                            on_wait=extra[i:i + 2], on_update=[])
                        out.append(ev)
                    inst.sync_info = bass_rust.SyncInfo(
                        on_wait=keep,
                        on_update=list(si.on_update) if si.on_update else [])
                    changed = True
                out.append(inst)
            if changed:
                blk.instructions = out
    return n


def _build_nc():
    nc = bass.Bass()

    x_d = nc.declare_dram_parameter("x", [T, C], F32, isOutput=False)
    wq_d = nc.declare_dram_parameter("wq", [C, CSLICE], F32, isOutput=False)
    wk_d = nc.declare_dram_parameter("wk", [C, CSLICE], F32, isOutput=False)
    wv_d = nc.declare_dram_parameter("wv", [C, CSLICE], F32, isOutput=False)
    wo_d = nc.declare_dram_parameter("wo", [G * 128, C], F32, isOutput=False)
    bq_d = nc.declare_dram_parameter("bq", [G, 128], F32, isOutput=False)
    bk_d = nc.declare_dram_parameter("bk", [G, 128], F32, isOutput=False)
    bv_d = nc.declare_dram_parameter("bv", [1, CSLICE], F32, isOutput=False)
    bo_d = nc.declare_dram_parameter("bo", [1, C], F32, isOutput=False)
    tri_d = nc.declare_dram_parameter("tri", [128, 128], BF16, isOutput=False)
    y_d = nc.declare_dram_parameter("y", [T, C], F32, isOutput=True)

    with tile.TileContext(nc) as tc, ExitStack() as ctx:
        consts = ctx.enter_context(tc.tile_pool(name="consts", bufs=1))
        stage = ctx.enter_context(tc.tile_pool(name="stage", bufs=3))
        epool = ctx.enter_context(tc.tile_pool(name="epool", bufs=6))
        small = ctx.enter_context(tc.tile_pool(name="small", bufs=4))

        # ---- constants ----
        tri = consts.tile([128, 128], BF16)
        nc.sync.dma_start(out=tri, in_=tri_d[:, :])
        eps_sb = consts.tile([128, 1], F32)
        nc.vector.memset(eps_sb, 1e-30)
        bq_sb = consts.tile([128, G], F32)
        nc.sync.dma_start(out=bq_sb, in_=bq_d.rearrange("g p -> p g"))
        bk_sb = consts.tile([128, G], F32)
        nc.sync.dma_start(out=bk_sb, in_=bk_d.rearrange("g p -> p g"))
        # free-dim-varying biases must be physically replicated across
        # partitions (DVE operands need nonzero partition step)
        bv_sb = consts.tile([128, CSLICE], F32)
        nc.sync.dma_start(out=bv_sb, in_=bv_d[0:1, :].to_broadcast((128, CSLICE)))
        bo_sb = consts.tile([128, C], F32)
        nc.sync.dma_start(out=bo_sb, in_=bo_d[0:1, :].to_broadcast((128, C)))

        # ---- weights: load fp32, cast to bf16 stationaries ----
        # wq/wk spaced: per group g, chunk cc: [128c, 128] with head j's 16
        # cols at free offset 32j.
        wq_sb = consts.tile([128, G, NCH, 128], BF16)
        wk_sb = consts.tile([128, G, NCH, 128], BF16)
        nc.vector.memset(wq_sb, 0.0)
        nc.vector.memset(wk_sb, 0.0)
        wv_sb = consts.tile([128, NCH, CSLICE], BF16)
        wo_sb = consts.tile([128, G, C], BF16)
        wqf = stage.tile([128, NCH, CSLICE], F32, tag="wstage")
        nc.sync.dma_start(out=wqf, in_=wq_d.rearrange("(a p) w -> p a w", p=128))
        wkf = stage.tile([128, NCH, CSLICE], F32, tag="wstage2")
        nc.sync.dma_start(out=wkf, in_=wk_d.rearrange("(a p) w -> p a w", p=128))
        wvf = stage.tile([128, NCH, CSLICE], F32, tag="wstage3")
        nc.sync.dma_start(out=wvf, in_=wv_d.rearrange("(a p) w -> p a w", p=128))
        for g in range(G):
            for j in range(4):
                h = 4 * g + j
                nc.vector.tensor_copy(
                    wq_sb[:, g, :, 32 * j:32 * j + HDIM],
                    wqf[:, :, HDIM * h:HDIM * (h + 1)])
                nc.vector.tensor_copy(
                    wk_sb[:, g, :, 32 * j:32 * j + HDIM],
                    wkf[:, :, HDIM * h:HDIM * (h + 1)])
        nc.vector.tensor_copy(wv_sb, wvf)
        for g in range(G):
            wof = stage.tile([128, C], F32, tag="wofull")
            nc.sync.dma_start(out=wof, in_=wo_d[g * 128:(g + 1) * 128, :])
            nc.vector.tensor_copy(wo_sb[:, g, :], wof)

        # ---- x: load, cast, transpose ----
        xT = consts.tile([128, NCH, T], BF16)   # xT[c, cc, t] = x[t, 128cc+c]
        for tt in range(NT):
            xs = stage.tile([128, C], F32, tag="xload")
            nc.sync.dma_start(out=xs, in_=x_d[tt * 128:(tt + 1) * 128, :])
            xb = stage.tile([128, C], BF16, tag="xcast")
            nc.vector.tensor_copy(xb, xs)
            # one chunk-level xbar transpose: out extra dim (cc) is
            # logically part of the partition dim -> [1024c, 128t]
            nc.sync.dma_start_transpose(
                out=xT[:, :, tt * 128:(tt + 1) * 128],
                in_=xb,
            )

        # ---- qkv projections ----
        qT = consts.tile([128, G, T], BF16)     # spaced: head j at part 32j
        kT = consts.tile([128, G, T], BF16)
        V = consts.tile([128, NT, HPC * 32], BF16)  # [t, tt, 8*32] (17 live cols
        # per head padded to 32 with zeros so packed PV matmuls write all
        # 128 PSUM partitions)
        nc.vector.memset(V, 0.0)

        # Single fixed PSUM footprint for the whole kernel: 3x2-bank score
        # sets + 2x1-bank accumulators (also used by qkv and out-proj) = 8.
        psco = ctx.enter_context(tc.tile_pool(name="psco", bufs=3, space="PSUM"))
        ppv = ctx.enter_context(tc.tile_pool(name="ppv", bufs=2, space="PSUM"))

        if True:
            for g in range(G):
                for qn in range(NQ):
                    ps_t = psco.tile([128, 1024], F32, tag="sset")
                    ps = ps_t[:, 0:512]
                    for cc in range(NCH):
                        nc.tensor.matmul(
                            out=ps, lhsT=wq_sb[:, g, cc, :],
                            rhs=xT[:, cc, qn * 512:(qn + 1) * 512],
                            start=(cc == 0), stop=(cc == NCH - 1),
                        )
                    nc.vector.tensor_scalar_add(
                        out=qT[:, g, qn * 512:(qn + 1) * 512], in0=ps,
                        scalar1=bq_sb[:, g:g + 1],
                    )
                    ps2_t = psco.tile([128, 1024], F32, tag="sset")
                    ps2 = ps2_t[:, 0:512]
                    for cc in range(NCH):
                        nc.tensor.matmul(
                            out=ps2, lhsT=wk_sb[:, g, cc, :],
                            rhs=xT[:, cc, qn * 512:(qn + 1) * 512],
                            start=(cc == 0), stop=(cc == NCH - 1),
                        )
                    nc.vector.tensor_scalar_add(
                        out=kT[:, g, qn * 512:(qn + 1) * 512], in0=ps2,
                        scalar1=bk_sb[:, g:g + 1],
                    )
            for tt in range(NT):
                ps_t = psco.tile([128, 1024], F32, tag="sset")
                ps = ps_t[:, 0:CSLICE]
                for cc in range(NCH):
                    nc.tensor.matmul(
                        out=ps, lhsT=xT[:, cc, tt * 128:(tt + 1) * 128],
                        rhs=wv_sb[:, cc, :],
                        start=(cc == 0), stop=(cc == NCH - 1),
                    )
                vdst = V[:, tt, :].rearrange("p (h e) -> p h e", h=HPC)[:, :, 0:HDIM]
                nc.vector.tensor_tensor(
                    vdst, ps.rearrange("p (h e) -> p h e", h=HPC),
                    bv_sb.rearrange("p (h e) -> p h e", h=HPC),
                    mybir.AluOpType.add,
                )
                ones_ap = V[:, tt, :].rearrange("p (h e) -> p h e", h=HPC)[:, :, HDIM:HDIM + 1]
                nc.vector.memset(ones_ap, 1.0)

        # ---- attention + output projection ----
        # group-spaced attn output: head j of group g at partitions
        # 32j..32j+15 of attnT[:, g, :]; rows 16..31 of each quadrant zero
        attnT = consts.tile([128, G, T], BF16)
        nc.vector.memset(attnT, 0.0)

        if True:
            for g in range(G):
                for qn in range(NQ):
                    pv = ppv.tile([128, 512], F32, tag="pv")
                    nkc = 4 * qn + 4
                    # two head-pairs sequentially: pair a covers j=2a, 2a+1.
                    # One 2-bank S-set per kc (bufs=3 -> scores run up to 3 kc
                    # ahead of exp, keeping ScalarE fed and PE dense/warm).
                    for a in range(2):
                        for kc in range(nkc):
                            sset = psco.tile([128, 1024], F32, tag="sset")
                            for jj in range(2):
                                j = 2 * a + jj
                                nc.tensor.matmul(
                                    out=sset[:, 512 * jj:512 * jj + 512],
                                    lhsT=kT[32 * j:32 * j + HDIM, g, kc * 128:(kc + 1) * 128],
                                    rhs=qT[32 * j:32 * j + HDIM, g, qn * 512:(qn + 1) * 512],
                                    start=True, stop=True,
                                    tile_position=(32 * j, 0),
                                )
                            et = epool.tile([128, 1024], BF16, tag="expT")
                            if kc >= 4 * qn:
                                # partial block: exp only f >= 128*jjj (valid
                                # region); memset the dead prefix; triangle-
                                # mask the diagonal stripe
                                jjj = kc - 4 * qn
                                er = et.rearrange("p (h q) -> p h q", h=2)
                                sr = sset.rearrange("p (h q) -> p h q", h=2)
                                nc.scalar.activation(
                                    out=er[:, :, 128 * jjj:512],
                                    in_=sr[:, :, 128 * jjj:512],
                                    func=AF.Exp, scale=0.25)
                                nc.vector.tensor_tensor(
                                    er[:, :, 128 * jjj:128 * jjj + 128],
                                    er[:, :, 128 * jjj:128 * jjj + 128],
                                    tri[:, None, :].to_broadcast((128, 2, 128)),
                                    mybir.AluOpType.mult,
                                )
                                if jjj > 0:
                                    nc.vector.memset(er[:, :, 0:128 * jjj], 0.0)
                            else:
                                nc.scalar.activation(out=et, in_=sset, func=AF.Exp, scale=0.25)
                            for jj in range(2):
                                j = 2 * a + jj
                                h = 4 * g + j
                                nc.tensor.matmul(
                                    out=pv[32 * j:32 * j + 32, :],
                                    lhsT=V[:, kc, 32 * h:32 * h + 32],
                                    rhs=et[:, 512 * jj:512 * jj + 512],
                                    start=(kc == 0), stop=(kc == nkc - 1),
                                    tile_position=(0, 32 * j),
                                    # sim group tracker is partition-base blind;
                                    # packed heads write disjoint partitions
                                    skip_group_check=True,
                                )
                    # normalize: 1/rowsum via exp(-0.5*ln(x^2+eps)) over the
                    # whole pv tile (unused lanes finite), then quadrant
                    # broadcast of each sum row.
                    sq_t = small.tile([128, 512], F32, tag="sqt")
                    nc.scalar.activation(out=sq_t, in_=pv, func=AF.Square)
                    ln_t = small.tile([128, 512], F32, tag="lnt")
                    nc.scalar.activation(out=ln_t, in_=sq_t, func=AF.Ln, bias=eps_sb[:, 0:1])
                    rec_t = small.tile([128, 512], F32, tag="rect")
                    nc.scalar.activation(out=rec_t, in_=ln_t, func=AF.Exp, scale=-0.5)
                    rec_rep = small.tile([128, 512], F32, tag="recrep")
                    nc.vector.stream_shuffle(rec_rep, rec_t, [HDIM] * 32)
                    for j in range(4):
                        nc.vector.tensor_tensor(
                            attnT[32 * j:32 * j + HDIM, g, qn * 512:(qn + 1) * 512],
                            pv[32 * j:32 * j + HDIM, :],
                            rec_rep[32 * j:32 * j + HDIM, :],
                            mybir.AluOpType.mult,
                        )

            # output projection (reuses ppv PSUM slots)
            for tt in range(NT):
                for nn in range(2):
                    ps = ppv.tile([128, 512], F32, tag="pv")
                    for g in range(G):
                        nc.tensor.matmul(
                            out=ps, lhsT=attnT[:, g, tt * 128:(tt + 1) * 128],
                            rhs=wo_sb[:, g, nn * 512:(nn + 1) * 512],
                            start=(g == 0), stop=(g == G - 1),
                        )
                    ys = stage.tile([128, 512], F32, tag="yout")
                    nc.vector.tensor_tensor(
                        ys, ps, bo_sb[:, nn * 512:(nn + 1) * 512],
                        mybir.AluOpType.add,
                    )
                    nc.sync.dma_start(
                        out=y_d[tt * 128:(tt + 1) * 128, nn * 512:(nn + 1) * 512],
                        in_=ys,
                    )
    return nc


def _make_in_maps(x, W_qkv, b_qkv, W_out, b_out):
    x2 = np.ascontiguousarray(np.asarray(x, dtype=np.float32).reshape(T, C))
    W_qkv = np.asarray(W_qkv, dtype=np.float32)
    b_qkv = np.asarray(b_qkv, dtype=np.float32)
    W_out = np.asarray(W_out, dtype=np.float32)
    b_out = np.asarray(b_out, dtype=np.float32)

    tri = np.zeros((128, 128), dtype=np.float32)
    for p in range(128):
        tri[p, p:] = 1.0
    tri = tri.astype(ml_dtypes.bfloat16)

    in_maps = []
    for p in range(NCORES):
        c0 = p * CSLICE
        wq = np.ascontiguousarray(W_qkv[:, c0:c0 + CSLICE])
        wk = np.ascontiguousarray(W_qkv[:, C + c0:C + c0 + CSLICE])
        wv = np.ascontiguousarray(W_qkv[:, 2 * C + c0:2 * C + c0 + CSLICE])
        # spaced W_out: row g*128 + 32j + d = W_out[c0 + 16*(4g+j) + d]
        wo = np.zeros((G * 128, C), dtype=np.float32)
        for g in range(G):
            for j in range(4):
                src_r = c0 + HDIM * (4 * g + j)
                wo[g * 128 + 32 * j:g * 128 + 32 * j + HDIM, :] = \
                    W_out[src_r:src_r + HDIM, :]
        bq = np.zeros((G, 128), dtype=np.float32)
        bk = np.zeros((G, 128), dtype=np.float32)
        for g in range(G):
            for j in range(4):
                h = 8 * p + 4 * g + j
                bq[g, 32 * j:32 * j + HDIM] = b_qkv[HDIM * h:HDIM * (h + 1)]
                bk[g, 32 * j:32 * j + HDIM] = b_qkv[C + HDIM * h:C + HDIM * (h + 1)]
        bv = np.ascontiguousarray(b_qkv[2 * C + c0:2 * C + c0 + CSLICE]).reshape(1, CSLICE)
        bo = (b_out if p == 0 else np.zeros_like(b_out)).reshape(1, C)
        in_maps.append({
            "x": x2, "wq": wq, "wk": wk, "wv": wv, "wo": wo,
            "bq": bq, "bk": bk, "bv": bv.astype(np.float32),
            "bo": bo.astype(np.float32), "tri": tri,
        })
    return in_maps


def kernel(x, attn_mask, W_qkv, b_qkv, W_out, b_out):
    if "nc" not in _CACHE:
        nc = _build_nc()
        _legalize_waits(nc)   # sim-incompatible but required by walrus
        _CACHE["nc"] = nc
    nc = _CACHE["nc"]
    in_maps = _make_in_maps(x, W_qkv, b_qkv, W_out, b_out)
    res = run_bass_kernel_spmd(nc, in_maps, core_ids=list(range(NCORES)))
    y = np.zeros((T, C), dtype=np.float32)
    for r in res.results:
        y += r["y"].astype(np.float32)
    return y.reshape(1, T, C)



# revision 11
# speedup vs baseline: 1.1161x; 1.1161x over previous
"""Causal multi-head attention kernel for Trainium2 (8 NeuronCores).

Problem: x[1,2048,1024] -> qkv proj (W_qkv[1024,3072]) -> 64 heads of dim 16
         -> causal softmax attention -> out proj (W_out[1024,1024]).

Sharding: Megatron-style head parallelism. Each of the 8 cores owns 8 heads
(a 128-wide column slice of W_qkv per q/k/v and a 128-row slice of W_out),
computes a partial output projection, and the host sums the 8 partials
(the "all-reduce").

v4 design (empirically grounded in traces of three prior variants):
  * PE matmul pairs sharing one PSUM tile DO execute concurrently (~600ns
    per 512-col pair under the sustained-power throttle); tiles coupled to
    per-head ACT reads serialize, and 4-way quads don't beat pairs.  So the
    inner loop keeps the proven (g, head-pair, kc) structure: 2 row-packed
    score matmuls -> one [128,1024] PSUM tile, one wide exp, 2 col-packed
    PV matmuls.
  * causal diagonal blocks: additive -1e30 triangle on the PSUM scores
    (DVE) before exp, and both score and PV matmuls TRIM the causally-dead
    query columns (~15% of attention streaming).  PSUM stop is a sim-only
    flag, so sub-range PV accumulation is safe on hardware.
  * qkv projections run DENSE bf16 (all 8 heads in 128 output partitions,
    no dead spaced columns -- halves qkv matmul count vs spaced).  Head
    groups are even heads (g=0, partitions 32j) and odd heads (g=1),
    extracted by one intra-quadrant stream_shuffle (-16 partition shift).
    fp8 qkv was tried and REVERTED: zero-mean random dot products keep the
    full per-element relative error (no 1/sqrt(N) averaging).
  * softmax 1/rowsum on DVE reciprocal (ScalarE does nothing but Exp);
    front-end (x DMA/cast/xbar-transpose + qkv) and the output projection
    of the previous block are woven as fillers into the attention loop.

Self-contained: hardcodes all shapes; host code only slices/prepacks inputs
per core and sums the 8 partial outputs.
"""

import numpy as np
from contextlib import ExitStack

import ml_dtypes

import concourse.bass as bass
import concourse.tile as tile
from concourse import mybir
from concourse.bass_utils import run_bass_kernel_spmd
import bass_rust

F32 = mybir.dt.float32
BF16 = mybir.dt.bfloat16
AF = mybir.ActivationFunctionType

T = 2048
C = 1024
HDIM = 16
NHEADS = 64
NCORES = 8
HPC = NHEADS // NCORES      # 8 heads per core
CSLICE = HPC * HDIM         # 128 channel slice per core
G = 2                       # head groups: g=0 even heads, g=1 odd heads
NCH = C // 128              # 8 contraction chunks
NT = T // 128               # 16 token chunks of 128
NQ = T // 512               # 4 query blocks of 512

ESCALE = 0.25               # softmax scale 1/sqrt(HDIM)
SHIFT16 = list(range(16, 32)) + list(range(16, 32))

_CACHE = {}


def _legalize_waits(nc):
    """This neuronxcc/walrus build encodes at most ONE sync-wait per
    instruction (two on EventSemaphore) -- multi-wait sync_info dies in
    codegen with "Too many sync wait commands".  Hoist excess waits into
    standalone EventSemaphore instructions on the same engine immediately
    before the instruction (engine queues are in-order, so semantics are
    preserved)."""
    n = 0
    for f in nc.m.functions:
        for blk in f.blocks:
            out = []
            changed = False
            for inst in blk.instructions:
                si = inst.sync_info
                waits = list(si.on_wait) if si is not None and si.on_wait else []
                cap = 2 if isinstance(inst, mybir.InstEventSemaphore) else 1
                if len(waits) > cap:
                    extra, keep = waits[:-cap], waits[-cap:]
                    for i in range(0, len(extra), 2):
                        ev = mybir.InstEventSemaphore(
                            name=f"evwait-{n}", ins=[], outs=[])
                        n += 1
                        ev.engine = inst.engine
                        ev.sync_info = bass_rust.SyncInfo(
                            on_wait=extra[i:i + 2], on_update=[])
                        out.append(ev)
                    inst.sync_info = bass_rust.SyncInfo(
                        on_wait=keep,
                        on_update=list(si.on_update) if si.on_update else [])
                    changed = True
                out.append(inst)
            if changed:
                blk.instructions = out
    return n


def _build_nc():
    nc = bass.Bass()

    x_d = nc.declare_dram_parameter("x", [T, C], F32, isOutput=False)
    wq_d = nc.declare_dram_parameter("wq", [128, NCH, 128], BF16, isOutput=False)
    wk_d = nc.declare_dram_parameter("wk", [128, NCH, 128], BF16, isOutput=False)
    wv_d = nc.declare_dram_parameter("wv", [128, NCH, 128], BF16, isOutput=False)
    wo_d = nc.declare_dram_parameter("wo", [G * 128, C], BF16, isOutput=False)
    bq_d = nc.declare_dram_parameter("bq", [128, 1], F32, isOutput=False)
    bk_d = nc.declare_dram_parameter("bk", [128, 1], F32, isOutput=False)
    bv_d = nc.declare_dram_parameter("bv", [128, 1], F32, isOutput=False)
    bo_d = nc.declare_dram_parameter("bo", [1, C], F32, isOutput=False)
    ntri_d = nc.declare_dram_parameter("ntri", [128, 128], BF16, isOutput=False)
    y_d = nc.declare_dram_parameter("y", [T, C], F32, isOutput=True)

    with tile.TileContext(nc) as tc, ExitStack() as ctx:
        consts = ctx.enter_context(tc.tile_pool(name="consts", bufs=1))
        stage = ctx.enter_context(tc.tile_pool(name="stage", bufs=3))
        epool = ctx.enter_context(tc.tile_pool(name="epool", bufs=6))
        small = ctx.enter_context(tc.tile_pool(name="small", bufs=4))
        psco = ctx.enter_context(tc.tile_pool(name="psco", bufs=3, space="PSUM"))
        ppv = ctx.enter_context(tc.tile_pool(name="ppv", bufs=2, space="PSUM"))

        # ---- constants (all host-prepped; DMA only) ----
        ntri = consts.tile([128, 128], BF16)   # 0 upper-tri / -1e30 lower
        nc.sync.dma_start(out=ntri, in_=ntri_d[:, :])
        bq_sb = consts.tile([128, 1], F32)
        nc.sync.dma_start(out=bq_sb, in_=bq_d[:, :])
        bk_sb = consts.tile([128, 1], F32)
        nc.sync.dma_start(out=bk_sb, in_=bk_d[:, :])
        bv_sb = consts.tile([128, 1], F32)
        nc.sync.dma_start(out=bv_sb, in_=bv_d[:, :])
        bo_sb = consts.tile([128, C], F32)
        nc.sync.dma_start(out=bo_sb, in_=bo_d[0:1, :].to_broadcast((128, C)))
        wq_sb = consts.tile([128, NCH, 128], BF16)
        nc.sync.dma_start(out=wq_sb, in_=wq_d[:, :, :])
        wk_sb = consts.tile([128, NCH, 128], BF16)
        nc.sync.dma_start(out=wk_sb, in_=wk_d[:, :, :])
        wv_sb = consts.tile([128, NCH, 128], BF16)
        nc.sync.dma_start(out=wv_sb, in_=wv_d[:, :, :])
        wo_sb = consts.tile([128, G, C], BF16)
        nc.sync.dma_start(out=wo_sb, in_=wo_d.rearrange("(g p) w -> p g w", g=G))

        # ---- persistent activations ----
        xT = consts.tile([128, NCH, T], BF16)    # xT[c, cc, t] = x[t, 128cc+c]
        qdn = consts.tile([128, T], BF16)        # dense: head h at parts 16h
        qod = consts.tile([128, T], BF16)        # odd heads shifted to 32j
        kdn = consts.tile([128, T], BF16)
        kod = consts.tile([128, T], BF16)
        # V[k, tt, h, 0:16] = v ; [..,16] = 1 (rowsum col); rest 0
        V = consts.tile([128, NT, HPC, 32], BF16)
        nc.vector.memset(V, 0.0)
        nc.vector.memset(V[:, :, :, HDIM:HDIM + 1], 1.0)
        attnT = consts.tile([128, G, T], BF16)   # head 2j+g at part 32j
        nc.vector.memset(attnT, 0.0)

        # ---- emission helpers ----
        def emit_xslice(tt):
            xs = stage.tile([128, C], F32, tag="xload")
            nc.sync.dma_start(out=xs, in_=x_d[tt * 128:(tt + 1) * 128, :])
            xb = stage.tile([128, C], BF16, tag="xcast")
            nc.vector.tensor_copy(xb, xs)
            nc.sync.dma_start_transpose(
                out=xT[:, :, tt * 128:(tt + 1) * 128], in_=xb)

        def emit_proj(qn, w_sb, b_sb, dn, od):
            q0 = qn * 512
            pt = psco.tile([128, 1024], F32, tag="sset",
                           name=f"proj_{qn}_{dn.name}")
            ps = pt[:, 0:512]
            for cc in range(NCH):
                nc.tensor.matmul(
                    out=ps, lhsT=w_sb[:, cc, :], rhs=xT[:, cc, q0:q0 + 512],
                    start=(cc == 0), stop=(cc == NCH - 1))
            nc.vector.tensor_scalar_add(
                out=dn[:, q0:q0 + 512], in0=ps, scalar1=b_sb[:, 0:1])
            if od is not None:
                nc.vector.stream_shuffle(
                    od[:, q0:q0 + 512], dn[:, q0:q0 + 512], SHIFT16)

        def emit_q(qn):
            emit_proj(qn, wq_sb, bq_sb, qdn, qod)

        def emit_k(qn):
            emit_proj(qn, wk_sb, bk_sb, kdn, kod)

        def emit_v(qn):
            q0 = qn * 512
            pt = psco.tile([128, 1024], F32, tag="sset", name=f"vproj_{qn}")
            ps = pt[:, 0:512]
            for cc in range(NCH):
                nc.tensor.matmul(
                    out=ps, lhsT=wv_sb[:, cc, :], rhs=xT[:, cc, q0:q0 + 512],
                    start=(cc == 0), stop=(cc == NCH - 1))
            vt = stage.tile([128, 512], BF16, tag="vt")
            nc.vector.tensor_scalar_add(out=vt, in0=ps, scalar1=bv_sb[:, 0:1])
            vtmp = stage.tile([128, 4, 128], BF16, tag="vtmp")
            nc.sync.dma_start_transpose(out=vtmp, in_=vt)
            nc.vector.tensor_copy(
                V[:, 4 * qn:4 * qn + 4, :, 0:HDIM],
                vtmp.rearrange("t a (h e) -> t a h e", h=HPC))

        def emit_outproj_unit(qn, ts, nn, pool):
            tok = qn * 512 + ts * 128
            if pool is psco:
                pt = psco.tile([128, 1024], F32, tag="sset",
                               name=f"op_{qn}_{ts}_{nn}")
                ps = pt[:, 0:512]
            else:
                ps = ppv.tile([128, 512], F32, tag="pv",
                              name=f"opv_{qn}_{ts}_{nn}")
            for g in range(G):
                nc.tensor.matmul(
                    out=ps, lhsT=attnT[:, g, tok:tok + 128],
                    rhs=wo_sb[:, g, nn * 512:(nn + 1) * 512],
                    start=(g == 0), stop=(g == G - 1))
            ys = stage.tile([128, 512], F32, tag="yout")
            nc.vector.tensor_tensor(
                ys, ps, bo_sb[:, nn * 512:(nn + 1) * 512],
                mybir.AluOpType.add)
            nc.sync.dma_start(
                out=y_d[tok:tok + 128, nn * 512:(nn + 1) * 512], in_=ys)

        def emit_normalize(qn, g, pv):
            q0 = qn * 512
            rec = small.tile([128, 512], F32, tag="rec")
            nc.vector.reciprocal(rec, pv)
            rep = small.tile([128, 512], F32, tag="recrep")
            nc.vector.stream_shuffle(rep, rec, [HDIM] * 32)
            for j in range(4):
                nc.vector.tensor_tensor(
                    attnT[32 * j:32 * j + HDIM, g, q0:q0 + 512],
                    pv[32 * j:32 * j + HDIM, :],
                    rep[32 * j:32 * j + HDIM, :],
                    mybir.AluOpType.mult)

        # ---- startup: first query block's inputs (exposed prologue) ----
        for tt in range(4):
            emit_xslice(tt)
        emit_q(0)
        emit_k(0)
        emit_v(0)

        # ---- main qn-pipelined loop ----
        for qn in range(NQ):
            q0 = qn * 512
            nkc = 4 * qn + 4
            fillers = []
            if qn > 0:
                for ts in range(4):
                    for nn in range(2):
                        fillers.append(
                            (emit_outproj_unit, (qn - 1, ts, nn, psco)))
            if qn < NQ - 1:
                for i in range(4):
                    fillers.append((emit_xslice, (4 * (qn + 1) + i,)))
                fillers.append((emit_q, (qn + 1,)))
                fillers.append((emit_k, (qn + 1,)))
                fillers.append((emit_v, (qn + 1,)))

            pvs = []
            for g in range(G):
                qsrc, ksrc = (qdn, kdn) if g == 0 else (qod, kod)
                pv = ppv.tile([128, 512], F32, tag="pv", name=f"pv_{qn}_{g}")
                pvs.append(pv)
                for a in range(2):
                    for kc in range(nkc):
                        diag = kc >= 4 * qn
                        jjj = kc - 4 * qn if diag else 0
                        c0 = 128 * jjj      # first causally-live query col
                        sset = psco.tile([128, 1024], F32, tag="sset",
                                         name=f"ss_{qn}_{g}_{a}_{kc}")
                        for jj in range(2):
                            j = 2 * a + jj
                            nc.tensor.matmul(
                                out=sset[:, 512 * jj + c0:512 * jj + 512],
                                lhsT=ksrc[32 * j:32 * j + HDIM,
                                          kc * 128:(kc + 1) * 128],
                                rhs=qsrc[32 * j:32 * j + HDIM,
                                         q0 + c0:q0 + 512],
                                start=True, stop=True,
                                tile_position=(32 * j, 0),
                            )
                        et = epool.tile([128, 1024], BF16, tag="expT",
                                        name=f"et_{qn}_{g}_{a}_{kc}")
                        er = et.rearrange("p (h q) -> p h q", h=2)
                        sr = sset.rearrange("p (h q) -> p h q", h=2)
                        if diag:
                            nc.vector.tensor_tensor(
                                sr[:, :, c0:c0 + 128],
                                sr[:, :, c0:c0 + 128],
                                ntri[:, None, :].to_broadcast((128, 2, 128)),
                                mybir.AluOpType.add)
                            nc.scalar.activation(
                                out=er[:, :, c0:512], in_=sr[:, :, c0:512],
                                func=AF.Exp, scale=ESCALE)
                        else:
                            nc.scalar.activation(out=et, in_=sset,
                                                 func=AF.Exp, scale=ESCALE)
                        for jj in range(2):
                            j = 2 * a + jj
                            h = 2 * j + g
                            nc.tensor.matmul(
                                out=pv[32 * j:32 * j + 32, c0:512],
                                lhsT=V[:, kc, h, :],
                                rhs=et[:, 512 * jj + c0:512 * jj + 512],
                                start=(kc == 0), stop=(kc == nkc - 1),
                                tile_position=(0, 32 * j),
                                skip_group_check=True,
                            )
                        if fillers:
                            fn, args = fillers.pop(0)
                            fn(*args)
            for g in range(G):
                emit_normalize(qn, g, pvs[g])
            while fillers:
                fn, args = fillers.pop(0)
                fn(*args)

        # ---- tail: last block's output projection ----
        for i, (ts, nn) in enumerate([(t, n) for t in range(4) for n in range(2)]):
            emit_outproj_unit(NQ - 1, ts, nn, psco if i % 2 == 0 else ppv)
    return nc


def _make_in_maps(x, W_qkv, b_qkv, W_out, b_out):
    x2 = np.ascontiguousarray(np.asarray(x, dtype=np.float32).reshape(T, C))
    W_qkv = np.asarray(W_qkv, dtype=np.float32)
    b_qkv = np.asarray(b_qkv, dtype=np.float32)
    W_out = np.asarray(W_out, dtype=np.float32)
    b_out = np.asarray(b_out, dtype=np.float32)

    ntri = np.full((128, 128), -1e30, dtype=np.float32)
    for p in range(128):
        ntri[p, p:] = 0.0
    ntri = ntri.astype(ml_dtypes.bfloat16)

    in_maps = []
    for p in range(NCORES):
        c0 = p * CSLICE

        def chunked(w):   # [1024, 128] -> [128, NCH, 128]
            return np.ascontiguousarray(
                w.reshape(NCH, 128, 128).transpose(1, 0, 2))

        wq = chunked(W_qkv[:, c0:c0 + CSLICE]).astype(ml_dtypes.bfloat16)
        wk = chunked(W_qkv[:, C + c0:C + c0 + CSLICE]).astype(ml_dtypes.bfloat16)
        wv = chunked(W_qkv[:, 2 * C + c0:2 * C + c0 + CSLICE]).astype(
            ml_dtypes.bfloat16)
        # spaced W_out rows follow attnT layout: head 2j+g at g*128+32j
        wo = np.zeros((G * 128, C), dtype=np.float32)
        for g in range(G):
            for j in range(4):
                src_r = c0 + HDIM * (2 * j + g)
                wo[g * 128 + 32 * j:g * 128 + 32 * j + HDIM, :] = \
                    W_out[src_r:src_r + HDIM, :]
        wo = wo.astype(ml_dtypes.bfloat16)
        bq = b_qkv[c0:c0 + CSLICE].reshape(128, 1).astype(np.float32)
        bk = b_qkv[C + c0:C + c0 + CSLICE].reshape(128, 1).astype(np.float32)
        bv = b_qkv[2 * C + c0:2 * C + c0 + CSLICE].reshape(128, 1).astype(
            np.float32)
        bo = (b_out if p == 0 else np.zeros_like(b_out)).reshape(1, C)
        in_maps.append({
            "x": x2, "wq": wq, "wk": wk, "wv": wv, "wo": wo,
            "bq": bq, "bk": bk, "bv": bv,
            "bo": bo.astype(np.float32), "ntri": ntri,
        })
    return in_maps


def kernel(x, attn_mask, W_qkv, b_qkv, W_out, b_out):
    if "nc" not in _CACHE:
        nc = _build_nc()
        _legalize_waits(nc)   # sim-incompatible but required by walrus
        _CACHE["nc"] = nc
    nc = _CACHE["nc"]
    in_maps = _make_in_maps(x, W_qkv, b_qkv, W_out, b_out)
    res = run_bass_kernel_spmd(nc, in_maps, core_ids=list(range(NCORES)))
    y = np.zeros((T, C), dtype=np.float32)
    for r in res.results:
        y += r["y"].astype(np.float32)
    return y.reshape(1, T, C)


# revision 14
# speedup vs baseline: 1.1851x; 1.0619x over previous
"""Causal multi-head attention kernel for Trainium2 (8 NeuronCores).

Problem: x[1,2048,1024] -> qkv proj (W_qkv[1024,3072]) -> 64 heads of dim 16
         -> causal softmax attention -> out proj (W_out[1024,1024]).

Sharding: Megatron-style head parallelism. Each of the 8 cores owns 8 heads
(a 128-wide column slice of W_qkv per q/k/v and a 128-row slice of W_out),
computes a partial output projection, and the host sums the 8 partials
(the "all-reduce").

v4 design (empirically grounded in traces of three prior variants):
  * PE matmul pairs sharing one PSUM tile DO execute concurrently (~600ns
    per 512-col pair under the sustained-power throttle); tiles coupled to
    per-head ACT reads serialize, and 4-way quads don't beat pairs.  So the
    inner loop keeps the proven (g, head-pair, kc) structure: 2 row-packed
    score matmuls -> one [128,1024] PSUM tile, one wide exp, 2 col-packed
    PV matmuls.
  * causal diagonal blocks: additive -1e30 triangle on the PSUM scores
    (DVE) before exp, and both score and PV matmuls TRIM the causally-dead
    query columns (~15% of attention streaming).  PSUM stop is a sim-only
    flag, so sub-range PV accumulation is safe on hardware.
  * qkv projections run DENSE bf16 (all 8 heads in 128 output partitions,
    no dead spaced columns -- halves qkv matmul count vs spaced).  Head
    groups are even heads (g=0, partitions 32j) and odd heads (g=1),
    extracted by one intra-quadrant stream_shuffle (-16 partition shift).
    fp8 qkv was tried and REVERTED: zero-mean random dot products keep the
    full per-element relative error (no 1/sqrt(N) averaging).
  * softmax 1/rowsum on DVE reciprocal (ScalarE does nothing but Exp);
    front-end (x DMA/cast/xbar-transpose + qkv) and the output projection
    of the previous block are woven as fillers into the attention loop.

Self-contained: hardcodes all shapes; host code only slices/prepacks inputs
per core and sums the 8 partial outputs.
"""

import numpy as np
from contextlib import ExitStack

import ml_dtypes

import concourse.bass as bass
import concourse.tile as tile
from concourse import mybir
from concourse.bass_utils import run_bass_kernel_spmd
import bass_rust

F32 = mybir.dt.float32
BF16 = mybir.dt.bfloat16
AF = mybir.ActivationFunctionType

T = 2048
C = 1024
HDIM = 16
NHEADS = 64
NCORES = 8
HPC = NHEADS // NCORES      # 8 heads per core
CSLICE = HPC * HDIM         # 128 channel slice per core
G = 2                       # head groups: g=0 even heads, g=1 odd heads
NCH = C // 128              # 8 contraction chunks
NT = T // 128               # 16 token chunks of 128
NQ = T // 512               # 4 query blocks of 512

ESCALE = 0.25               # softmax scale 1/sqrt(HDIM)
SHIFT16 = list(range(16, 32)) + list(range(16, 32))

_CACHE = {}


def _legalize_waits(nc):
    """This neuronxcc/walrus build encodes at most ONE sync-wait per
    instruction (two on EventSemaphore) -- multi-wait sync_info dies in
    codegen with "Too many sync wait commands".  Hoist excess waits into
    standalone EventSemaphore instructions on the same engine immediately
    before the instruction (engine queues are in-order, so semantics are
    preserved)."""
    n = 0
    for f in nc.m.functions:
        for blk in f.blocks:
            out = []
            changed = False
            for inst in blk.instructions:
                si = inst.sync_info
                waits = list(si.on_wait) if si is not None and si.on_wait else []
                cap = 2 if isinstance(inst, mybir.InstEventSemaphore) else 1
                if len(waits) > cap:
                    extra, keep = waits[:-cap], waits[-cap:]
                    for i in range(0, len(extra), 2):
                        ev = mybir.InstEventSemaphore(
                            name=f"evwait-{n}", ins=[], outs=[])
                        n += 1
                        ev.engine = inst.engine
                        ev.sync_info = bass_rust.SyncInfo(
                            on_wait=extra[i:i + 2], on_update=[])
                        out.append(ev)
                    inst.sync_info = bass_rust.SyncInfo(
                        on_wait=keep,
                        on_update=list(si.on_update) if si.on_update else [])
                    changed = True
                out.append(inst)
            if changed:
                blk.instructions = out
    return n


def _build_nc():
    nc = bass.Bass()

    x_d = nc.declare_dram_parameter("x", [T, C], F32, isOutput=False)
    wq_d = nc.declare_dram_parameter("wq", [128, NCH, 128], BF16, isOutput=False)
    wk_d = nc.declare_dram_parameter("wk", [128, NCH, 128], BF16, isOutput=False)
    wv_d = nc.declare_dram_parameter("wv", [128, NCH, 128], BF16, isOutput=False)
    wo_d = nc.declare_dram_parameter("wo", [G * 128, C], BF16, isOutput=False)
    bq_d = nc.declare_dram_parameter("bq", [128, 1], F32, isOutput=False)
    bk_d = nc.declare_dram_parameter("bk", [128, 1], F32, isOutput=False)
    bv_d = nc.declare_dram_parameter("bv", [128, 1], F32, isOutput=False)
    bo_d = nc.declare_dram_parameter("bo", [1, C], F32, isOutput=False)
    ntri_d = nc.declare_dram_parameter("ntri", [128, 128], BF16, isOutput=False)
    y_d = nc.declare_dram_parameter("y", [T, C], F32, isOutput=True)

    with tile.TileContext(nc) as tc, ExitStack() as ctx:
        consts = ctx.enter_context(tc.tile_pool(name="consts", bufs=1))
        stage = ctx.enter_context(tc.tile_pool(name="stage", bufs=3))
        epool = ctx.enter_context(tc.tile_pool(name="epool", bufs=6))
        small = ctx.enter_context(tc.tile_pool(name="small", bufs=4))
        psco = ctx.enter_context(tc.tile_pool(name="psco", bufs=3, space="PSUM"))
        ppv = ctx.enter_context(tc.tile_pool(name="ppv", bufs=2, space="PSUM"))

        # ---- constants (all host-prepped; DMA only) ----
        ntri = consts.tile([128, 128], BF16)   # 0 upper-tri / -1e30 lower
        nc.sync.dma_start(out=ntri, in_=ntri_d[:, :])
        bq_sb = consts.tile([128, 1], F32)
        nc.sync.dma_start(out=bq_sb, in_=bq_d[:, :])
        bk_sb = consts.tile([128, 1], F32)
        nc.sync.dma_start(out=bk_sb, in_=bk_d[:, :])
        bv_sb = consts.tile([128, 1], F32)
        nc.sync.dma_start(out=bv_sb, in_=bv_d[:, :])
        wq_sb = consts.tile([128, NCH, 128], BF16)
        nc.sync.dma_start(out=wq_sb, in_=wq_d[:, :, :])
        wk_sb = consts.tile([128, NCH, 128], BF16)
        nc.sync.dma_start(out=wk_sb, in_=wk_d[:, :, :])
        wv_sb = consts.tile([128, NCH, 128], BF16)
        nc.sync.dma_start(out=wv_sb, in_=wv_d[:, :, :])
        # wo/bo are first needed by out-proj fillers a whole block later;
        # their DMAs are emitted after the startup qkv (emission order =
        # sync-queue order, so they must not delay the x chunks)
        bo_sb = consts.tile([128, C], F32)
        wo_sb = consts.tile([128, G, C], BF16)

        # ---- persistent activations ----
        xT = consts.tile([128, NCH, T], BF16)    # xT[c, cc, t] = x[t, 128cc+c]
        qdn = consts.tile([128, T], BF16)        # dense: head h at parts 16h
        qod = consts.tile([128, T], BF16)        # odd heads shifted to 32j
        kdn = consts.tile([128, T], BF16)
        kod = consts.tile([128, T], BF16)
        # V[k, tt, h, 0:16] = v ; [..,16] = 1 (rowsum col); rest 0
        V = consts.tile([128, NT, HPC, 32], BF16)
        nc.vector.memset(V, 0.0)
        nc.vector.memset(V[:, :, :, HDIM:HDIM + 1], 1.0)
        attnT = consts.tile([128, G, T], BF16)   # head 2j+g at part 32j
        nc.vector.memset(attnT, 0.0)

        # ---- emission helpers ----
        def emit_xslice(tt):
            xs = stage.tile([128, C], F32, tag="xload")
            nc.sync.dma_start(out=xs, in_=x_d[tt * 128:(tt + 1) * 128, :])
            xb = stage.tile([128, C], BF16, tag="xcast")
            nc.vector.tensor_copy(xb, xs)
            nc.sync.dma_start_transpose(
                out=xT[:, :, tt * 128:(tt + 1) * 128], in_=xb)

        def emit_proj(qn, w_sb, b_sb, dn, od):
            q0 = qn * 512
            pt = psco.tile([128, 1024], F32, tag="sset",
                           name=f"proj_{qn}_{dn.name}")
            ps = pt[:, 0:512]
            for cc in range(NCH):
                nc.tensor.matmul(
                    out=ps, lhsT=w_sb[:, cc, :], rhs=xT[:, cc, q0:q0 + 512],
                    start=(cc == 0), stop=(cc == NCH - 1))
            nc.vector.tensor_scalar_add(
                out=dn[:, q0:q0 + 512], in0=ps, scalar1=b_sb[:, 0:1])
            if od is not None:
                nc.vector.stream_shuffle(
                    od[:, q0:q0 + 512], dn[:, q0:q0 + 512], SHIFT16)

        def emit_q(qn):
            emit_proj(qn, wq_sb, bq_sb, qdn, qod)

        def emit_k(qn):
            emit_proj(qn, wk_sb, bk_sb, kdn, kod)

        def emit_v(qn):
            q0 = qn * 512
            pt = psco.tile([128, 1024], F32, tag="sset", name=f"vproj_{qn}")
            ps = pt[:, 0:512]
            for cc in range(NCH):
                nc.tensor.matmul(
                    out=ps, lhsT=wv_sb[:, cc, :], rhs=xT[:, cc, q0:q0 + 512],
                    start=(cc == 0), stop=(cc == NCH - 1))
            vt = stage.tile([128, 512], BF16, tag="vt")
            nc.vector.tensor_scalar_add(out=vt, in0=ps, scalar1=bv_sb[:, 0:1])
            vtmp = stage.tile([128, 4, 128], BF16, tag="vtmp")
            nc.sync.dma_start_transpose(out=vtmp, in_=vt)
            nc.vector.tensor_copy(
                V[:, 4 * qn:4 * qn + 4, :, 0:HDIM],
                vtmp.rearrange("t a (h e) -> t a h e", h=HPC))

        def emit_outproj_unit(qn, ts, nn, pool):
            tok = qn * 512 + ts * 128
            if pool is psco:
                pt = psco.tile([128, 1024], F32, tag="sset",
                               name=f"op_{qn}_{ts}_{nn}")
                ps = pt[:, 0:512]
            else:
                ps = ppv.tile([128, 512], F32, tag="pv",
                              name=f"opv_{qn}_{ts}_{nn}")
            for g in range(G):
                nc.tensor.matmul(
                    out=ps, lhsT=attnT[:, g, tok:tok + 128],
                    rhs=wo_sb[:, g, nn * 512:(nn + 1) * 512],
                    start=(g == 0), stop=(g == G - 1))
            ys = stage.tile([128, 512], F32, tag="yout")
            nc.vector.tensor_tensor(
                ys, ps, bo_sb[:, nn * 512:(nn + 1) * 512],
                mybir.AluOpType.add)
            nc.sync.dma_start(
                out=y_d[tok:tok + 128, nn * 512:(nn + 1) * 512], in_=ys)

        def emit_normalize(qn, g, pv):
            q0 = qn * 512
            rec = small.tile([128, 512], F32, tag="rec")
            nc.vector.reciprocal(rec, pv)
            rep = small.tile([128, 512], F32, tag="recrep")
            nc.vector.stream_shuffle(rep, rec, [HDIM] * 32)
            for j in range(4):
                nc.vector.tensor_tensor(
                    attnT[32 * j:32 * j + HDIM, g, q0:q0 + 512],
                    pv[32 * j:32 * j + HDIM, :],
                    rep[32 * j:32 * j + HDIM, :],
                    mybir.AluOpType.mult)

        # ---- startup: first query block's inputs (exposed prologue) ----
        for tt in range(4):
            emit_xslice(tt)
        emit_q(0)
        emit_k(0)
        emit_v(0)
        nc.sync.dma_start(out=bo_sb, in_=bo_d[0:1, :].to_broadcast((128, C)))
        nc.sync.dma_start(out=wo_sb, in_=wo_d.rearrange("(g p) w -> p g w", g=G))

        # ---- main qn-pipelined loop ----
        # out-proj of block qn runs as fillers of block qn+2 (attnT persists,
        # and the early blocks are already PE-oversubscribed)
        for qn in range(NQ):
            q0 = qn * 512
            nkc = 4 * qn + 4
            fillers = []
            if qn < NQ - 1:
                for i in range(4):
                    fillers.append((emit_xslice, (4 * (qn + 1) + i,)))
                fillers.append((emit_q, (qn + 1,)))
                fillers.append((emit_k, (qn + 1,)))
                fillers.append((emit_v, (qn + 1,)))
            for back in (2, 1) if qn == NQ - 1 else (2,):
                if qn - back >= 0:
                    for ts in range(4):
                        for nn in range(2):
                            fillers.append(
                                (emit_outproj_unit, (qn - back, ts, nn, psco)))

            pvs = []
            for g in range(G):
                qsrc, ksrc = (qdn, kdn) if g == 0 else (qod, kod)
                pv = ppv.tile([128, 512], F32, tag="pv", name=f"pv_{qn}_{g}")
                pvs.append(pv)
                for a in range(2):
                    for kc in range(nkc):
                        diag = kc >= 4 * qn
                        jjj = kc - 4 * qn if diag else 0
                        c0 = 128 * jjj      # first causally-live query col
                        sset = psco.tile([128, 1024], F32, tag="sset",
                                         name=f"ss_{qn}_{g}_{a}_{kc}")
                        for jj in range(2):
                            j = 2 * a + jj
                            nc.tensor.matmul(
                                out=sset[:, 512 * jj + c0:512 * jj + 512],
                                lhsT=ksrc[32 * j:32 * j + HDIM,
                                          kc * 128:(kc + 1) * 128],
                                rhs=qsrc[32 * j:32 * j + HDIM,
                                         q0 + c0:q0 + 512],
                                start=True, stop=True,
                                tile_position=(32 * j, 0),
                            )
                        et = epool.tile([128, 1024], BF16, tag="expT",
                                        name=f"et_{qn}_{g}_{a}_{kc}")
                        er = et.rearrange("p (h q) -> p h q", h=2)
                        sr = sset.rearrange("p (h q) -> p h q", h=2)
                        if diag:
                            nc.vector.tensor_tensor(
                                sr[:, :, c0:c0 + 128],
                                sr[:, :, c0:c0 + 128],
                                ntri[:, None, :].to_broadcast((128, 2, 128)),
                                mybir.AluOpType.add)
                            nc.scalar.activation(
                                out=er[:, :, c0:512], in_=sr[:, :, c0:512],
                                func=AF.Exp, scale=ESCALE)
                        else:
                            nc.scalar.activation(out=et, in_=sset,
                                                 func=AF.Exp, scale=ESCALE)
                        for jj in range(2):
                            j = 2 * a + jj
                            h = 2 * j + g
                            nc.tensor.matmul(
                                out=pv[32 * j:32 * j + 32, c0:512],
                                lhsT=V[:, kc, h, :],
                                rhs=et[:, 512 * jj + c0:512 * jj + 512],
                                start=(kc == 0), stop=(kc == nkc - 1),
                                tile_position=(0, 32 * j),
                                skip_group_check=True,
                            )
                        if fillers:
                            fn, args = fillers.pop(0)
                            fn(*args)
            for g in range(G):
                emit_normalize(qn, g, pvs[g])
            while fillers:
                fn, args = fillers.pop(0)
                fn(*args)

        # ---- tail: last block's output projection (5-deep psum rotation) ----
        for i, (ts, nn) in enumerate([(t, n) for t in range(4) for n in range(2)]):
            emit_outproj_unit(NQ - 1, ts, nn, psco if i % 5 < 3 else ppv)
    return nc


def _make_in_maps(x, W_qkv, b_qkv, W_out, b_out):
    x2 = np.ascontiguousarray(np.asarray(x, dtype=np.float32).reshape(T, C))
    W_qkv = np.asarray(W_qkv, dtype=np.float32)
    b_qkv = np.asarray(b_qkv, dtype=np.float32)
    W_out = np.asarray(W_out, dtype=np.float32)
    b_out = np.asarray(b_out, dtype=np.float32)

    ntri = np.full((128, 128), -1e30, dtype=np.float32)
    for p in range(128):
        ntri[p, p:] = 0.0
    ntri = ntri.astype(ml_dtypes.bfloat16)

    in_maps = []
    for p in range(NCORES):
        c0 = p * CSLICE

        def chunked(w):   # [1024, 128] -> [128, NCH, 128]
            return np.ascontiguousarray(
                w.reshape(NCH, 128, 128).transpose(1, 0, 2))

        wq = chunked(W_qkv[:, c0:c0 + CSLICE]).astype(ml_dtypes.bfloat16)
        wk = chunked(W_qkv[:, C + c0:C + c0 + CSLICE]).astype(ml_dtypes.bfloat16)
        wv = chunked(W_qkv[:, 2 * C + c0:2 * C + c0 + CSLICE]).astype(
            ml_dtypes.bfloat16)
        # spaced W_out rows follow attnT layout: head 2j+g at g*128+32j
        wo = np.zeros((G * 128, C), dtype=np.float32)
        for g in range(G):
            for j in range(4):
                src_r = c0 + HDIM * (2 * j + g)
                wo[g * 128 + 32 * j:g * 128 + 32 * j + HDIM, :] = \
                    W_out[src_r:src_r + HDIM, :]
        wo = wo.astype(ml_dtypes.bfloat16)
        bq = b_qkv[c0:c0 + CSLICE].reshape(128, 1).astype(np.float32)
        bk = b_qkv[C + c0:C + c0 + CSLICE].reshape(128, 1).astype(np.float32)
        bv = b_qkv[2 * C + c0:2 * C + c0 + CSLICE].reshape(128, 1).astype(
            np.float32)
        bo = (b_out if p == 0 else np.zeros_like(b_out)).reshape(1, C)
        in_maps.append({
            "x": x2, "wq": wq, "wk": wk, "wv": wv, "wo": wo,
            "bq": bq, "bk": bk, "bv": bv,
            "bo": bo.astype(np.float32), "ntri": ntri,
        })
    return in_maps


def kernel(x, attn_mask, W_qkv, b_qkv, W_out, b_out):
    if "nc" not in _CACHE:
        nc = _build_nc()
        _legalize_waits(nc)   # sim-incompatible but required by walrus
        _CACHE["nc"] = nc
    nc = _CACHE["nc"]
    in_maps = _make_in_maps(x, W_qkv, b_qkv, W_out, b_out)
    res = run_bass_kernel_spmd(nc, in_maps, core_ids=list(range(NCORES)))
    y = np.zeros((T, C), dtype=np.float32)
    for r in res.results:
        y += r["y"].astype(np.float32)
    return y.reshape(1, T, C)
